# revision 4
# baseline (speedup 1.0000x reference)
"""GraphSAGE (4-layer) on 8 Trainium2 NeuronCores via Bass/Tile.

Sharding: nodes partitioned across 8 cores by destination (graph parallel).
Each core owns 6250 nodes (padded to 6272 = 49*128) and the edges whose dst
lands in its range. Per layer, the gather table (node features, or h@wn for
the matmul-first layer) is AllGathered so every core can gather arbitrary
source rows; the gather-mean is computed as one-hot matmuls on the
TensorEngine over dst-sorted edge tiles.

Aggregation is linear, so it always runs at width 128 for layers 1-3:
  L1: agg(feat)         @ w1n   (aggregate-first, 128 wide)
  L2: agg(h1 @ w2n)              (matmul-first, 128 wide)
  L3: agg(h2)           @ w3n   (aggregate-first, 128 wide)
  L4: agg(h3)           @ w4n   (256 wide)

Self-contained: hardcodes all shapes; builds + compiles the Bass program at
call time from the actual edge lists.
"""

import sys

sys.path.insert(0, "/opt/trn_rl_repo")

import numpy as np

import concourse.bacc as bacc
import concourse.bass as bass
import concourse.mybir as mybir
import concourse.tile as tile
from concourse.bass_utils import run_bass_kernel_spmd
from concourse.masks import make_identity

P = 128
NCORES = 8
N_NODES = 50000
PER = 6250  # real nodes per core
ROWS = 6272  # padded rows per core (49 * 128)
NB = ROWS // P  # 49 dst blocks per core
NPAD = NCORES * ROWS  # 50176 table rows
HALF = NPAD // 2  # 25088 rows per int16-addressable half
ZR = HALF - 1  # zero row index within each half (core 3/7 pad row 6271)
F_IN = 128
H = 256
MID = 128
F32 = mybir.dt.float32
I16 = mybir.dt.int16

# blocks per gather chunk, by gather width
CH_128 = 2
CH_256 = 1


def _preprocess(feat, src, dst):
    """Host-side sharding. Returns per-core input maps + common structure."""
    src = np.asarray(src).astype(np.int64)
    dst = np.asarray(dst).astype(np.int64)
    feat = np.asarray(feat).astype(np.float32)

    deg = np.bincount(dst, minlength=N_NODES).astype(np.float32)
    inv_deg = (1.0 / np.maximum(deg, 1.0)).astype(np.float32)

    src_row = (src // PER) * ROWS + (src % PER)
    half = (src_row // HALF).astype(np.int64)
    idx16 = (src_row % HALF).astype(np.int64)
    dst_core = dst // PER
    dst_local = dst % PER
    blk = dst_local // P
    ldst = dst_local % P

    # per-core, per-(block, half) edge counts
    counts = np.zeros((NCORES, NB, 2), np.int64)
    for c in range(NCORES):
        m = dst_core == c
        key = blk[m] * 2 + half[m]
        counts[c] = np.bincount(key, minlength=NB * 2).reshape(NB, 2)
    n_sub = np.maximum(1, -(-counts.max(axis=0) // P))  # [NB, 2] subtiles, >=1

    sub_base = np.zeros((NB, 2), np.int64)  # subtile offset within each half stream
    sub_base[1:, 0] = np.cumsum(n_sub[:-1, 0])
    sub_base[1:, 1] = np.cumsum(n_sub[:-1, 1])
    L_half = [int(n_sub[:, h].sum()) * P for h in range(2)]  # idxs per half
    col_base = np.zeros(NB, np.int64)  # S-stream column offset per block
    col_base[1:] = np.cumsum(n_sub.sum(axis=1))[:-1]
    S_TOT = int(n_sub.sum())

    in_maps = []
    for c in range(NCORES):
        m = np.nonzero(dst_core == c)[0]
        b_c, h_c, i_c, l_c = blk[m], half[m], idx16[m], ldst[m]
        order = np.lexsort((h_c, b_c))
        b_c, h_c, i_c, l_c = b_c[order], h_c[order], i_c[order], l_c[order]
        # rank within each (b, h) group
        key = b_c * 2 + h_c
        grp_cnt = np.bincount(key, minlength=NB * 2)
        grp_start = np.concatenate([[0], np.cumsum(grp_cnt)[:-1]])
        rank = np.arange(len(key)) - grp_start[key]
        s_idx = rank // P
        lane = rank % P

        idx_streams = []
        for h in range(2):
            arr = np.full(L_half[h], ZR, np.int64)
            sel = h_c == h
            pos = (sub_base[b_c[sel], h] + s_idx[sel]) * P + lane[sel]
            arr[pos] = i_c[sel]
            packed = arr.reshape(-1, 16).T.astype(np.int16)  # [16, L/16]
            idx_streams.append(np.tile(packed, (8, 1)).copy())

        ldst_arr = np.zeros((P, S_TOT), np.float32)
        col = col_base[b_c] + np.where(h_c == 0, s_idx, n_sub[b_c, 0] + s_idx)
        ldst_arr[lane, col] = l_c.astype(np.float32)

        feat_fm = np.zeros((P, ROWS), np.float32)
        feat_fm[:, :PER] = feat[c * PER : (c + 1) * PER].T
        inv_own = np.ones(ROWS, np.float32)
        inv_own[:PER] = inv_deg[c * PER : (c + 1) * PER]
        invdeg_rep = np.tile(inv_own[None, :], (P, 1)).copy()

        in_maps.append(
            {
                "idx0": idx_streams[0],
                "idx1": idx_streams[1],
                "ldst": ldst_arr,
                "feat_fm": feat_fm,
                "invdeg": invdeg_rep,
            }
        )

    feat_nm = np.zeros((NPAD, F_IN), np.float32)
    rows_all = (np.arange(N_NODES) // PER) * ROWS + (np.arange(N_NODES) % PER)
    feat_nm[rows_all] = feat
    iota = np.tile(np.arange(P, dtype=np.float32)[None, :], (P, 1)).copy()
    for im in in_maps:
        im["feat_nm"] = feat_nm
        im["iota"] = iota

    structure = dict(n_sub=n_sub, sub_base=sub_base, col_base=col_base, L_half=L_half, S_TOT=S_TOT)
    return in_maps, structure, rows_all


def _build(structure):
    n_sub = structure["n_sub"]
    sub_base = structure["sub_base"]
    col_base = structure["col_base"]
    L_half = structure["L_half"]
    S_TOT = structure["S_TOT"]

    nc = bacc.Bacc("TRN2", target_bir_lowering=False, debug=False, num_devices=NCORES)

    # --- I/O ---
    feat_nm = nc.dram_tensor("feat_nm", [NPAD, F_IN], F32, kind="ExternalInput")
    feat_fm_in = nc.dram_tensor("feat_fm", [P, ROWS], F32, kind="ExternalInput")
    idx_in = [
        nc.dram_tensor(f"idx{h}", [P, L_half[h] // 16], I16, kind="ExternalInput")
        for h in range(2)
    ]
    ldst_in = nc.dram_tensor("ldst", [P, S_TOT], F32, kind="ExternalInput")
    invdeg_in = nc.dram_tensor("invdeg", [P, ROWS], F32, kind="ExternalInput")
    iota_in = nc.dram_tensor("iota", [P, P], F32, kind="ExternalInput")
    w_in = {}
    for name, shape in [
        ("w1s", [F_IN, H]), ("w1n", [F_IN, H]),
        ("w2s", [H, MID]), ("w2n", [H, MID]),
        ("w3s", [MID, H]), ("w3n", [MID, H]),
        ("w4s", [H, H]), ("w4n", [H, H]),
    ]:
        w_in[name] = nc.dram_tensor(name, shape, F32, kind="ExternalInput")
    b_in = {
        "b1": nc.dram_tensor("b1", [P, H // P], F32, kind="ExternalInput"),
        "b2": nc.dram_tensor("b2", [P, MID // P], F32, kind="ExternalInput"),
        "b3": nc.dram_tensor("b3", [P, H // P], F32, kind="ExternalInput"),
        "b4": nc.dram_tensor("b4", [P, H // P], F32, kind="ExternalInput"),
    }
    out = nc.dram_tensor("out", [ROWS, H], F32, kind="ExternalOutput")

    with tile.TileContext(nc) as tc:
        with (
            tc.tile_pool(name="const", bufs=1) as cpool,
            tc.tile_pool(name="hfm", bufs=4) as hpool,
            tc.tile_pool(name="gath", bufs=2) as gpool,
            tc.tile_pool(name="s", bufs=4) as spool,
            tc.tile_pool(name="small", bufs=3) as smpool,
            tc.tile_pool(name="pa", bufs=2, space="PSUM") as ppool_out,
            tc.tile_pool(name="pagg", bufs=2, space="PSUM") as ppool_agg,
            tc.tile_pool(name="dram", bufs=1, space="DRAM") as dpool,
        ):
            # ---- constants / weights to SBUF ----
            ident = cpool.tile([P, P], F32, name="ident")
            make_identity(nc, ident[:])
            iota_t = cpool.tile([P, P], F32, name="iota_t")
            nc.sync.dma_start(iota_t[:], iota_in[:])
            ldst_t = cpool.tile([P, S_TOT], F32, name="ldst_t")
            nc.sync.dma_start(ldst_t[:], ldst_in[:])

            def load_w(name, kparts):
                """weight [K, M] split into kparts SBUF tiles of [128, M]."""
                tiles = []
                for k in range(kparts):
                    t = cpool.tile(
                        [P, w_in[name].shape[1]], F32, name=f"{name}_{k}"
                    )
                    nc.sync.dma_start(t[:], w_in[name][k * P : (k + 1) * P, :])
                    tiles.append(t)
                return tiles

            w = {
                "w1s": load_w("w1s", 1), "w1n": load_w("w1n", 1),
                "w2s": load_w("w2s", 2), "w2n": load_w("w2n", 2),
                "w3s": load_w("w3s", 1), "w3n": load_w("w3n", 1),
                "w4s": load_w("w4s", 2), "w4n": load_w("w4n", 2),
            }
            bia = {}
            for name, d in [("b1", H), ("b2", MID), ("b3", H), ("b4", H)]:
                t = cpool.tile([P, d // P], F32, name=f"{name}_t")
                nc.sync.dma_start(t[:], b_in[name][:])
                bia[name] = t

            # ---- DRAM scratch ----
            t2_own = dpool.tile([ROWS, MID], F32, name="t2_own")
            t2_full = dpool.tile([NPAD, MID], F32, addr_space="Shared", name="t2_full")
            t3_own = dpool.tile([ROWS, MID], F32, name="t3_own")
            t3_full = dpool.tile([NPAD, MID], F32, addr_space="Shared", name="t3_full")
            t4_own = dpool.tile([ROWS, H], F32, name="t4_own")
            t4_full = dpool.tile([NPAD, H], F32, addr_space="Shared", name="t4_full")

            # ---- persistent h (feature-major [128, ROWS]) ----
            feat_fm = hpool.tile([P, ROWS], F32, tag="hfm", name="feat_fm_t")
            nc.sync.dma_start(feat_fm[:], feat_fm_in[:])
            h1 = [hpool.tile([P, ROWS], F32, tag="hfm", name=f"h1_{i}") for i in range(2)]
            h2 = [hpool.tile([P, ROWS], F32, tag="hfm", name="h2_0")]
            h3 = [hpool.tile([P, ROWS], F32, tag="hfm", name=f"h3_{i}") for i in range(2)]

            def gather_chunk(table_half_aps, b0, nblk, fw, lname):
                """Gather all subtiles for blocks [b0, b0+nblk) of both halves.
                Returns (tiles, sub0) where tiles[h] is [128, nsub_h, fw]."""
                tiles = []
                for h in range(2):
                    ns = int(n_sub[b0 : b0 + nblk, h].sum())
                    g = gpool.tile(
                        [P, ns, fw], F32, tag=f"g{h}", name=f"g{lname}_{b0}_{h}"
                    )
                    nidx = ns * P
                    off16 = int(sub_base[b0, h]) * P // 16
                    ix = spool.tile(
                        [P, nidx // 16], I16, tag=f"ix{h}", name=f"ix{lname}_{b0}_{h}"
                    )
                    nc.sync.dma_start(ix[:], idx_in[h][:, off16 : off16 + nidx // 16])
                    # ucode is only stable up to ~384 idxs per call
                    for s0 in range(0, ns, 3):
                        sn = min(3, ns - s0)
                        nc.gpsimd.dma_gather(
                            out_ap=g[:, s0 : s0 + sn, :],
                            in_ap=table_half_aps[h],
                            idxs_ap=ix[:, s0 * 8 : (s0 + sn) * 8],
                            num_idxs=sn * P,
                            num_idxs_reg=sn * P,
                            elem_size=fw,
                            queue_num=0,
                        )
                    tiles.append(g)
                return tiles

            def scatter_block(gtiles, b0, b, fw, lname):
                """One-hot matmul aggregation for dst block b (chunk origin b0).
                Returns list of PSUM tiles [128, 128], one per 128-feat slice."""
                nfh = fw // P
                aggs = [
                    ppool_agg.tile(
                        [P, P], F32, tag=f"agg{fh}",
                        bufs=(2 if fh == 0 else 1),
                        name=f"agg{lname}_{b}_{fh}",
                    )
                    for fh in range(nfh)
                ]
                subs = []  # (half, local subtile index)
                for h in range(2):
                    for s in range(int(n_sub[b, h])):
                        subs.append((h, s))
                nsub_tot = len(subs)
                for j, (h, s) in enumerate(subs):
                    # subtile index within the chunk's gather tile
                    sloc = int(n_sub[b0:b, h].sum()) + s
                    col = int(col_base[b]) + (s if h == 0 else int(n_sub[b, 0]) + s)
                    S = spool.tile([P, P], F32, tag="S", name=f"S{lname}_{b}_{j}")
                    nc.vector.tensor_tensor(
                        out=S[:],
                        in0=ldst_t[:, col : col + 1].to_broadcast([P, P]),
                        in1=iota_t[:],
                        op=mybir.AluOpType.is_equal,
                    )
                    for fh in range(nfh):
                        nc.tensor.matmul(
                            aggs[fh][:],
                            lhsT=gtiles[h][:, sloc, fh * P : (fh + 1) * P],
                            rhs=S[:],
                            start=(j == 0),
                            stop=(j == nsub_tot - 1),
                        )
                return aggs

            def layer(lname, table_aps, fw, wn, ws, h_prev, bias, dout, relu, h_out):
                """One GraphSAGE layer, dst-block streamed."""
                ch = CH_128 if fw == P else CH_256
                ndh = dout // P
                nkh = len(ws)  # K chunks of self path
                for b0 in range(0, NB, ch):
                    nblk = min(ch, NB - b0)
                    gtiles = gather_chunk(table_aps, b0, nblk, fw, lname)
                    for b in range(b0, b0 + nblk):
                        aggs = scatter_block(gtiles, b0, b, fw, lname)
                        # inv_deg scale: PSUM -> SBUF
                        invd = smpool.tile([P, P], F32, tag="invd", name=f"iv{lname}_{b}")
                        nc.sync.dma_start(invd[:], invdeg_in[:, b * P : (b + 1) * P])
                        aggs_s = []
                        for fh in range(len(aggs)):
                            a_s = smpool.tile(
                                [P, P], F32, tag=f"aggs{fh}", name=f"as{lname}_{b}_{fh}"
                            )
                            nc.vector.tensor_tensor(
                                out=a_s[:], in0=aggs[fh][:], in1=invd[:],
                                op=mybir.AluOpType.mult,
                            )
                            aggs_s.append(a_s)
                        for dh in range(ndh):
                            po = ppool_out.tile(
                                [P, P], F32, tag=f"out{dh}", name=f"po{lname}_{b}_{dh}"
                            )
                            first = True
                            # neighbor term
                            if wn is None:  # identity add (L2)
                                nc.tensor.matmul(
                                    po[:], lhsT=ident[:], rhs=aggs_s[0][:],
                                    start=first, stop=False,
                                )
                                first = False
                            else:
                                for fh in range(len(aggs_s)):
                                    nc.tensor.matmul(
                                        po[:],
                                        lhsT=wn[fh][:, dh * P : (dh + 1) * P],
                                        rhs=aggs_s[fh][:],
                                        start=first, stop=False,
                                    )
                                    first = False
                            # self term
                            for kh in range(nkh):
                                nc.tensor.matmul(
                                    po[:],
                                    lhsT=ws[kh][:, dh * P : (dh + 1) * P],
                                    rhs=h_prev[kh][:, b * P : (b + 1) * P],
                                    start=False, stop=(kh == nkh - 1),
                                )
                            dst_ap = h_out[dh][:, b * P : (b + 1) * P]
                            if relu:
                                nc.scalar.activation(
                                    dst_ap, po[:],
                                    mybir.ActivationFunctionType.Relu,
                                    bias=bias[:, dh : dh + 1],
                                )
                            else:
                                nc.vector.tensor_scalar_add(
                                    dst_ap, po[:], bias[:, dh : dh + 1]
                                )
                # zero the 22 pad columns so gather-table pad rows stay zero
                for dh in range(ndh):
                    nc.vector.memset(h_out[dh][:, PER:ROWS], 0.0)

            def allgather(own, full):
                nc.gpsimd.collective_compute(
                    "AllGather",
                    mybir.AluOpType.bypass,
                    replica_groups=[list(range(NCORES))],
                    ins=[own[:]],
                    outs=[full[:]],
                )

            def halves(t, fw):
                return [t[h * HALF : (h + 1) * HALF, :] for h in range(2)]

            # ================= Layer 1 =================
            layer(
                "l1", halves(feat_nm, F_IN), F_IN,
                wn=w["w1n"], ws=w["w1s"], h_prev=[feat_fm],
                bias=bia["b1"], dout=H, relu=True, h_out=h1,
            )

            # T2 = h1 @ w2n  (node-major), AllGather
            for rb in range(NB):
                pt = ppool_out.tile([P, MID], F32, tag="out0", name=f"t2_{rb}")
                for kh in range(2):
                    nc.tensor.matmul(
                        pt[:],
                        lhsT=h1[kh][:, rb * P : (rb + 1) * P],
                        rhs=w["w2n"][kh][:],
                        start=(kh == 0), stop=(kh == 1),
                    )
                st = smpool.tile([P, MID], F32, tag="stage", name=f"t2s_{rb}")
                nc.vector.tensor_copy(st[:], pt[:])
                nc.sync.dma_start(t2_own[rb * P : (rb + 1) * P, :], st[:])
            allgather(t2_own, t2_full)

            # ================= Layer 2 =================
            layer(
                "l2", halves(t2_full, MID), MID,
                wn=None, ws=w["w2s"], h_prev=h1,
                bias=bia["b2"], dout=MID, relu=True, h_out=h2,
            )

            # T3 = h2 (node-major via PE transpose), AllGather
            for rb in range(NB):
                pt = ppool_out.tile([P, P], F32, tag="out0", name=f"t3_{rb}")
                nc.tensor.transpose(pt[:], h2[0][:, rb * P : (rb + 1) * P], ident[:])
                st = smpool.tile([P, MID], F32, tag="stage", name=f"t3s_{rb}")
                nc.vector.tensor_copy(st[:], pt[:])
                nc.sync.dma_start(t3_own[rb * P : (rb + 1) * P, :], st[:])
            allgather(t3_own, t3_full)

            # ================= Layer 3 =================
            layer(
                "l3", halves(t3_full, MID), MID,
                wn=w["w3n"], ws=w["w3s"], h_prev=h2,
                bias=bia["b3"], dout=H, relu=True, h_out=h3,
            )

            # T4 = h3 (node-major via PE transpose), AllGather
            for rb in range(NB):
                st = smpool.tile([P, H], F32, tag="stage2", name=f"t4s_{rb}")
                for kh in range(2):
                    pt = ppool_out.tile([P, P], F32, tag=f"out{kh}", name=f"t4_{rb}_{kh}")
                    nc.tensor.transpose(
                        pt[:], h3[kh][:, rb * P : (rb + 1) * P], ident[:]
                    )
                    nc.vector.tensor_copy(st[:, kh * P : (kh + 1) * P], pt[:])
                nc.sync.dma_start(t4_own[rb * P : (rb + 1) * P, :], st[:])
            allgather(t4_own, t4_full)

            # ================= Layer 4 =================
            h4 = [hpool.tile([P, ROWS], F32, tag="hfm", name=f"h4_{i}") for i in range(2)]
            layer(
                "l4", halves(t4_full, H), H,
                wn=w["w4n"], ws=w["w4s"], h_prev=h3,
                bias=bia["b4"], dout=H, relu=False, h_out=h4,
            )

            # out = h4 transposed to node-major
            for rb in range(NB):
                st = smpool.tile([P, H], F32, tag="stage2", name=f"o_{rb}")
                for kh in range(2):
                    pt = ppool_out.tile([P, P], F32, tag=f"out{kh}", name=f"o_{rb}_{kh}")
                    nc.tensor.transpose(
                        pt[:], h4[kh][:, rb * P : (rb + 1) * P], ident[:]
                    )
                    nc.vector.tensor_copy(st[:, kh * P : (kh + 1) * P], pt[:])
                nc.sync.dma_start(out[rb * P : (rb + 1) * P, :], st[:])

    nc.compile()
    return nc


_CACHE = {}


def _run(inputs, trace=False):
    feat = inputs["feat"]
    in_maps, structure, rows_all = _preprocess(feat, inputs["src"], inputs["dst"])
    for im in in_maps:
        for i in (1, 2, 3, 4):
            im[f"w{i}s"] = np.asarray(inputs[f"w{i}s"]).astype(np.float32)
            im[f"w{i}n"] = np.asarray(inputs[f"w{i}n"]).astype(np.float32)
            im[f"b{i}"] = (
                np.asarray(inputs[f"b{i}"]).astype(np.float32).reshape(-1, P).T.copy()
            )

    key = "nc"
    if key not in _CACHE:
        _CACHE[key] = _build(structure)
    nc = _CACHE[key]

    res = run_bass_kernel_spmd(
        nc, in_maps, core_ids=list(range(NCORES)), trace=trace
    )
    outs = [res.results[c]["out"] for c in range(NCORES)]
    full = np.concatenate(outs, axis=0)  # [NPAD, H]
    result = full[rows_all]
    return result, res


def kernel(**inputs) -> np.ndarray:
    result, _ = _run(inputs, trace=False)
    return result


# revision 5
# speedup vs baseline: 1.0034x; 1.0034x over previous
"""GraphSAGE (4-layer) on 8 Trainium2 NeuronCores via Bass/Tile.

Sharding: nodes partitioned across 8 cores by destination (graph parallel).
Each core owns 6250 nodes (padded to 6272 = 49*128) and the edges whose dst
lands in its range. Per layer, the gather table (node features, or h@wn for
the matmul-first layer) is AllGathered so every core can gather arbitrary
source rows; the gather-mean is computed as one-hot matmuls on the
TensorEngine over dst-sorted edge tiles.

Aggregation is linear, so it always runs at width 128 for layers 1-3:
  L1: agg(feat)         @ w1n   (aggregate-first, 128 wide)
  L2: agg(h1 @ w2n)              (matmul-first, 128 wide)
  L3: agg(h2)           @ w3n   (aggregate-first, 128 wide)
  L4: agg(h3)           @ w4n   (256 wide)

Self-contained: hardcodes all shapes; builds + compiles the Bass program at
call time from the actual edge lists.
"""

import sys

sys.path.insert(0, "/opt/trn_rl_repo")

import numpy as np

import concourse.bacc as bacc
import concourse.bass as bass
import concourse.mybir as mybir
import concourse.tile as tile
from concourse.bass_utils import run_bass_kernel_spmd
from concourse.masks import make_identity

P = 128
NCORES = 8
N_NODES = 50000
PER = 6250  # real nodes per core
ROWS = 6272  # padded rows per core (49 * 128)
NB = ROWS // P  # 49 dst blocks per core
NPAD = NCORES * ROWS  # 50176 table rows
HALF = NPAD // 2  # 25088 rows per int16-addressable half
ZR = HALF - 1  # zero row index within each half (core 3/7 pad row 6271)
F_IN = 128
H = 256
MID = 128
F32 = mybir.dt.float32
I16 = mybir.dt.int16

# blocks per gather chunk, by gather width
CH_128 = 2
CH_256 = 1


def _preprocess(feat, src, dst):
    """Host-side sharding. Returns per-core input maps + common structure."""
    src = np.asarray(src).astype(np.int64)
    dst = np.asarray(dst).astype(np.int64)
    feat = np.asarray(feat).astype(np.float32)

    deg = np.bincount(dst, minlength=N_NODES).astype(np.float32)
    inv_deg = (1.0 / np.maximum(deg, 1.0)).astype(np.float32)

    src_row = (src // PER) * ROWS + (src % PER)
    half = (src_row // HALF).astype(np.int64)
    idx16 = (src_row % HALF).astype(np.int64)
    dst_core = dst // PER
    dst_local = dst % PER
    blk = dst_local // P
    ldst = dst_local % P

    # per-core, per-(block, half) edge counts
    counts = np.zeros((NCORES, NB, 2), np.int64)
    for c in range(NCORES):
        m = dst_core == c
        key = blk[m] * 2 + half[m]
        counts[c] = np.bincount(key, minlength=NB * 2).reshape(NB, 2)
    n_sub = np.maximum(1, -(-counts.max(axis=0) // P))  # [NB, 2] subtiles, >=1

    sub_base = np.zeros((NB, 2), np.int64)  # subtile offset within each half stream
    sub_base[1:, 0] = np.cumsum(n_sub[:-1, 0])
    sub_base[1:, 1] = np.cumsum(n_sub[:-1, 1])
    L_half = [int(n_sub[:, h].sum()) * P for h in range(2)]  # idxs per half
    col_base = np.zeros(NB, np.int64)  # S-stream column offset per block
    col_base[1:] = np.cumsum(n_sub.sum(axis=1))[:-1]
    S_TOT = int(n_sub.sum())

    in_maps = []
    for c in range(NCORES):
        m = np.nonzero(dst_core == c)[0]
        b_c, h_c, i_c, l_c = blk[m], half[m], idx16[m], ldst[m]
        order = np.lexsort((h_c, b_c))
        b_c, h_c, i_c, l_c = b_c[order], h_c[order], i_c[order], l_c[order]
        # rank within each (b, h) group
        key = b_c * 2 + h_c
        grp_cnt = np.bincount(key, minlength=NB * 2)
        grp_start = np.concatenate([[0], np.cumsum(grp_cnt)[:-1]])
        rank = np.arange(len(key)) - grp_start[key]
        s_idx = rank // P
        lane = rank % P

        idx_streams = []
        for h in range(2):
            arr = np.full(L_half[h], ZR, np.int64)
            sel = h_c == h
            pos = (sub_base[b_c[sel], h] + s_idx[sel]) * P + lane[sel]
            arr[pos] = i_c[sel]
            packed = arr.reshape(-1, 16).T.astype(np.int16)  # [16, L/16]
            idx_streams.append(np.tile(packed, (8, 1)).copy())

        ldst_arr = np.zeros((P, S_TOT), np.float32)
        col = col_base[b_c] + np.where(h_c == 0, s_idx, n_sub[b_c, 0] + s_idx)
        ldst_arr[lane, col] = l_c.astype(np.float32)

        feat_fm = np.zeros((P, ROWS), np.float32)
        feat_fm[:, :PER] = feat[c * PER : (c + 1) * PER].T
        inv_own = np.ones(ROWS, np.float32)
        inv_own[:PER] = inv_deg[c * PER : (c + 1) * PER]
        invdeg_rep = np.tile(inv_own[None, :], (P, 1)).copy()

        in_maps.append(
            {
                "idx0": idx_streams[0],
                "idx1": idx_streams[1],
                "ldst": ldst_arr,
                "feat_fm": feat_fm,
                "invdeg": invdeg_rep,
            }
        )

    feat_nm = np.zeros((NPAD, F_IN), np.float32)
    rows_all = (np.arange(N_NODES) // PER) * ROWS + (np.arange(N_NODES) % PER)
    feat_nm[rows_all] = feat
    maxsub = int(n_sub.sum(axis=1).max())
    iota = np.tile(np.arange(P, dtype=np.float32)[None, :], (P, maxsub)).copy()
    for im in in_maps:
        im["feat_nm"] = feat_nm
        im["iota"] = iota

    structure = dict(n_sub=n_sub, sub_base=sub_base, col_base=col_base, L_half=L_half, S_TOT=S_TOT, maxsub=maxsub)
    return in_maps, structure, rows_all


def _build(structure):
    n_sub = structure["n_sub"]
    sub_base = structure["sub_base"]
    col_base = structure["col_base"]
    L_half = structure["L_half"]
    S_TOT = structure["S_TOT"]

    nc = bacc.Bacc("TRN2", target_bir_lowering=False, debug=False, num_devices=NCORES)

    # --- I/O ---
    feat_nm = nc.dram_tensor("feat_nm", [NPAD, F_IN], F32, kind="ExternalInput")
    feat_fm_in = nc.dram_tensor("feat_fm", [P, ROWS], F32, kind="ExternalInput")
    idx_in = [
        nc.dram_tensor(f"idx{h}", [P, L_half[h] // 16], I16, kind="ExternalInput")
        for h in range(2)
    ]
    ldst_in = nc.dram_tensor("ldst", [P, S_TOT], F32, kind="ExternalInput")
    invdeg_in = nc.dram_tensor("invdeg", [P, ROWS], F32, kind="ExternalInput")
    MAXSUB = structure["maxsub"]
    iota_in = nc.dram_tensor("iota", [P, MAXSUB * P], F32, kind="ExternalInput")
    w_in = {}
    for name, shape in [
        ("w1s", [F_IN, H]), ("w1n", [F_IN, H]),
        ("w2s", [H, MID]), ("w2n", [H, MID]),
        ("w3s", [MID, H]), ("w3n", [MID, H]),
        ("w4s", [H, H]), ("w4n", [H, H]),
    ]:
        w_in[name] = nc.dram_tensor(name, shape, F32, kind="ExternalInput")
    b_in = {
        "b1": nc.dram_tensor("b1", [P, H // P], F32, kind="ExternalInput"),
        "b2": nc.dram_tensor("b2", [P, MID // P], F32, kind="ExternalInput"),
        "b3": nc.dram_tensor("b3", [P, H // P], F32, kind="ExternalInput"),
        "b4": nc.dram_tensor("b4", [P, H // P], F32, kind="ExternalInput"),
    }
    out = nc.dram_tensor("out", [ROWS, H], F32, kind="ExternalOutput")

    with tile.TileContext(nc) as tc:
        with (
            tc.tile_pool(name="const", bufs=1) as cpool,
            tc.tile_pool(name="hfm", bufs=4) as hpool,
            tc.tile_pool(name="gath", bufs=2) as gpool,
            tc.tile_pool(name="s", bufs=4) as spool,
            tc.tile_pool(name="small", bufs=3) as smpool,
            tc.tile_pool(name="pa", bufs=2, space="PSUM") as ppool_out,
            tc.tile_pool(name="pagg", bufs=2, space="PSUM") as ppool_agg,
            tc.tile_pool(name="dram", bufs=1, space="DRAM") as dpool,
        ):
            # ---- constants / weights to SBUF ----
            ident = cpool.tile([P, P], F32, name="ident")
            make_identity(nc, ident[:])
            iota_t = cpool.tile([P, MAXSUB * P], F32, name="iota_t")
            nc.sync.dma_start(iota_t[:], iota_in[:])
            ldst_t = cpool.tile([P, S_TOT], F32, name="ldst_t")
            nc.sync.dma_start(ldst_t[:], ldst_in[:])

            def load_w(name, kparts):
                """weight [K, M] split into kparts SBUF tiles of [128, M]."""
                tiles = []
                for k in range(kparts):
                    t = cpool.tile(
                        [P, w_in[name].shape[1]], F32, name=f"{name}_{k}"
                    )
                    nc.sync.dma_start(t[:], w_in[name][k * P : (k + 1) * P, :])
                    tiles.append(t)
                return tiles

            w = {
                "w1s": load_w("w1s", 1), "w1n": load_w("w1n", 1),
                "w2s": load_w("w2s", 2), "w2n": load_w("w2n", 2),
                "w3s": load_w("w3s", 1), "w3n": load_w("w3n", 1),
                "w4s": load_w("w4s", 2), "w4n": load_w("w4n", 2),
            }
            bia = {}
            for name, d in [("b1", H), ("b2", MID), ("b3", H), ("b4", H)]:
                t = cpool.tile([P, d // P], F32, name=f"{name}_t")
                nc.sync.dma_start(t[:], b_in[name][:])
                bia[name] = t

            # ---- DRAM scratch ----
            t2_own = dpool.tile([ROWS, MID], F32, name="t2_own")
            t2_full = dpool.tile([NPAD, MID], F32, addr_space="Shared", name="t2_full")
            t3_own = dpool.tile([ROWS, MID], F32, name="t3_own")
            t3_full = dpool.tile([NPAD, MID], F32, addr_space="Shared", name="t3_full")
            t4_own = dpool.tile([ROWS, H], F32, name="t4_own")
            t4_full = dpool.tile([NPAD, H], F32, addr_space="Shared", name="t4_full")

            # ---- persistent h (feature-major [128, ROWS]) ----
            feat_fm = hpool.tile([P, ROWS], F32, tag="hfm", name="feat_fm_t")
            nc.sync.dma_start(feat_fm[:], feat_fm_in[:])
            h1 = [hpool.tile([P, ROWS], F32, tag="hfm", name=f"h1_{i}") for i in range(2)]
            h2 = [hpool.tile([P, ROWS], F32, tag="hfm", name="h2_0")]
            h3 = [hpool.tile([P, ROWS], F32, tag="hfm", name=f"h3_{i}") for i in range(2)]

            def gather_chunk(table_half_aps, b0, nblk, fw, lname):
                """Gather all subtiles for blocks [b0, b0+nblk) of both halves.
                Returns (tiles, sub0) where tiles[h] is [128, nsub_h, fw]."""
                tiles = []
                for h in range(2):
                    ns = int(n_sub[b0 : b0 + nblk, h].sum())
                    g = gpool.tile(
                        [P, ns, fw], F32, tag=f"g{h}", name=f"g{lname}_{b0}_{h}"
                    )
                    nidx = ns * P
                    off16 = int(sub_base[b0, h]) * P // 16
                    ix = spool.tile(
                        [P, nidx // 16], I16, tag=f"ix{h}", name=f"ix{lname}_{b0}_{h}"
                    )
                    nc.sync.dma_start(ix[:], idx_in[h][:, off16 : off16 + nidx // 16])
                    # ucode is only stable up to ~384 idxs per call
                    for s0 in range(0, ns, 3):
                        sn = min(3, ns - s0)
                        nc.gpsimd.dma_gather(
                            out_ap=g[:, s0 : s0 + sn, :],
                            in_ap=table_half_aps[h],
                            idxs_ap=ix[:, s0 * 8 : (s0 + sn) * 8],
                            num_idxs=sn * P,
                            num_idxs_reg=sn * P,
                            elem_size=fw,
                            queue_num=0,
                        )
                    tiles.append(g)
                return tiles

            def scatter_block(gtiles, b0, b, fw, lname):
                """One-hot matmul aggregation for dst block b (chunk origin b0).
                Returns list of PSUM tiles [128, 128], one per 128-feat slice."""
                nfh = fw // P
                aggs = [
                    ppool_agg.tile(
                        [P, P], F32, tag=f"agg{fh}",
                        bufs=(2 if fh == 0 else 1),
                        name=f"agg{lname}_{b}_{fh}",
                    )
                    for fh in range(nfh)
                ]
                subs = []  # (half, local subtile index)
                for h in range(2):
                    for s in range(int(n_sub[b, h])):
                        subs.append((h, s))
                nsub_tot = len(subs)
                cb = int(col_base[b])
                S = spool.tile(
                    [P, nsub_tot, P], F32, tag="S", bufs=2, name=f"S{lname}_{b}"
                )
                nc.vector.tensor_tensor(
                    out=S[:],
                    in0=ldst_t[:, cb : cb + nsub_tot].to_broadcast([P, nsub_tot, P]),
                    in1=iota_t[:, : nsub_tot * P],
                    op=mybir.AluOpType.is_equal,
                )
                for j, (h, s) in enumerate(subs):
                    # subtile index within the chunk's gather tile
                    sloc = int(n_sub[b0:b, h].sum()) + s
                    for fh in range(nfh):
                        nc.tensor.matmul(
                            aggs[fh][:],
                            lhsT=gtiles[h][:, sloc, fh * P : (fh + 1) * P],
                            rhs=S[:, j, :],
                            start=(j == 0),
                            stop=(j == nsub_tot - 1),
                        )
                return aggs

            def layer(lname, table_aps, fw, wn, ws, h_prev, bias, dout, relu, h_out):
                """One GraphSAGE layer, dst-block streamed."""
                ch = CH_128 if fw == P else CH_256
                ndh = dout // P
                nkh = len(ws)  # K chunks of self path
                for b0 in range(0, NB, ch):
                    nblk = min(ch, NB - b0)
                    gtiles = gather_chunk(table_aps, b0, nblk, fw, lname)
                    for b in range(b0, b0 + nblk):
                        aggs = scatter_block(gtiles, b0, b, fw, lname)
                        # inv_deg scale: PSUM -> SBUF
                        invd = smpool.tile([P, P], F32, tag="invd", name=f"iv{lname}_{b}")
                        nc.sync.dma_start(invd[:], invdeg_in[:, b * P : (b + 1) * P])
                        aggs_s = []
                        for fh in range(len(aggs)):
                            a_s = smpool.tile(
                                [P, P], F32, tag=f"aggs{fh}", name=f"as{lname}_{b}_{fh}"
                            )
                            nc.vector.tensor_tensor(
                                out=a_s[:], in0=aggs[fh][:], in1=invd[:],
                                op=mybir.AluOpType.mult,
                            )
                            aggs_s.append(a_s)
                        for dh in range(ndh):
                            po = ppool_out.tile(
                                [P, P], F32, tag=f"out{dh}", name=f"po{lname}_{b}_{dh}"
                            )
                            first = True
                            # neighbor term
                            if wn is None:  # identity add (L2)
                                nc.tensor.matmul(
                                    po[:], lhsT=ident[:], rhs=aggs_s[0][:],
                                    start=first, stop=False,
                                )
                                first = False
                            else:
                                for fh in range(len(aggs_s)):
                                    nc.tensor.matmul(
                                        po[:],
                                        lhsT=wn[fh][:, dh * P : (dh + 1) * P],
                                        rhs=aggs_s[fh][:],
                                        start=first, stop=False,
                                    )
                                    first = False
                            # self term
                            for kh in range(nkh):
                                nc.tensor.matmul(
                                    po[:],
                                    lhsT=ws[kh][:, dh * P : (dh + 1) * P],
                                    rhs=h_prev[kh][:, b * P : (b + 1) * P],
                                    start=False, stop=(kh == nkh - 1),
                                )
                            dst_ap = h_out[dh][:, b * P : (b + 1) * P]
                            if relu:
                                nc.scalar.activation(
                                    dst_ap, po[:],
                                    mybir.ActivationFunctionType.Relu,
                                    bias=bias[:, dh : dh + 1],
                                )
                            else:
                                nc.vector.tensor_scalar_add(
                                    dst_ap, po[:], bias[:, dh : dh + 1]
                                )
                # zero the 22 pad columns so gather-table pad rows stay zero
                for dh in range(ndh):
                    nc.vector.memset(h_out[dh][:, PER:ROWS], 0.0)

            def allgather(own, full):
                nc.gpsimd.collective_compute(
                    "AllGather",
                    mybir.AluOpType.bypass,
                    replica_groups=[list(range(NCORES))],
                    ins=[own[:]],
                    outs=[full[:]],
                )

            def halves(t, fw):
                return [t[h * HALF : (h + 1) * HALF, :] for h in range(2)]

            # ================= Layer 1 =================
            layer(
                "l1", halves(feat_nm, F_IN), F_IN,
                wn=w["w1n"], ws=w["w1s"], h_prev=[feat_fm],
                bias=bia["b1"], dout=H, relu=True, h_out=h1,
            )

            # T2 = h1 @ w2n  (node-major), AllGather
            for rb in range(NB):
                pt = ppool_out.tile([P, MID], F32, tag="out0", name=f"t2_{rb}")
                for kh in range(2):
                    nc.tensor.matmul(
                        pt[:],
                        lhsT=h1[kh][:, rb * P : (rb + 1) * P],
                        rhs=w["w2n"][kh][:],
                        start=(kh == 0), stop=(kh == 1),
                    )
                st = smpool.tile([P, MID], F32, tag="stage", name=f"t2s_{rb}")
                nc.vector.tensor_copy(st[:], pt[:])
                nc.sync.dma_start(t2_own[rb * P : (rb + 1) * P, :], st[:])
            allgather(t2_own, t2_full)

            # ================= Layer 2 =================
            layer(
                "l2", halves(t2_full, MID), MID,
                wn=None, ws=w["w2s"], h_prev=h1,
                bias=bia["b2"], dout=MID, relu=True, h_out=h2,
            )

            # T3 = h2 (node-major via PE transpose), AllGather
            for rb in range(NB):
                pt = ppool_out.tile([P, P], F32, tag="out0", name=f"t3_{rb}")
                nc.tensor.transpose(pt[:], h2[0][:, rb * P : (rb + 1) * P], ident[:])
                st = smpool.tile([P, MID], F32, tag="stage", name=f"t3s_{rb}")
                nc.vector.tensor_copy(st[:], pt[:])
                nc.sync.dma_start(t3_own[rb * P : (rb + 1) * P, :], st[:])
            allgather(t3_own, t3_full)

            # ================= Layer 3 =================
            layer(
                "l3", halves(t3_full, MID), MID,
                wn=w["w3n"], ws=w["w3s"], h_prev=h2,
                bias=bia["b3"], dout=H, relu=True, h_out=h3,
            )

            # T4 = h3 (node-major via PE transpose), AllGather
            for rb in range(NB):
                st = smpool.tile([P, H], F32, tag="stage2", name=f"t4s_{rb}")
                for kh in range(2):
                    pt = ppool_out.tile([P, P], F32, tag=f"out{kh}", name=f"t4_{rb}_{kh}")
                    nc.tensor.transpose(
                        pt[:], h3[kh][:, rb * P : (rb + 1) * P], ident[:]
                    )
                    nc.vector.tensor_copy(st[:, kh * P : (kh + 1) * P], pt[:])
                nc.sync.dma_start(t4_own[rb * P : (rb + 1) * P, :], st[:])
            allgather(t4_own, t4_full)

            # ================= Layer 4 =================
            h4 = [hpool.tile([P, ROWS], F32, tag="hfm", name=f"h4_{i}") for i in range(2)]
            layer(
                "l4", halves(t4_full, H), H,
                wn=w["w4n"], ws=w["w4s"], h_prev=h3,
                bias=bia["b4"], dout=H, relu=False, h_out=h4,
            )

            # out = h4 transposed to node-major
            for rb in range(NB):
                st = smpool.tile([P, H], F32, tag="stage2", name=f"o_{rb}")
                for kh in range(2):
                    pt = ppool_out.tile([P, P], F32, tag=f"out{kh}", name=f"o_{rb}_{kh}")
                    nc.tensor.transpose(
                        pt[:], h4[kh][:, rb * P : (rb + 1) * P], ident[:]
                    )
                    nc.vector.tensor_copy(st[:, kh * P : (kh + 1) * P], pt[:])
                nc.sync.dma_start(out[rb * P : (rb + 1) * P, :], st[:])

    nc.compile()
    return nc


_CACHE = {}


def _run(inputs, trace=False):
    feat = inputs["feat"]
    in_maps, structure, rows_all = _preprocess(feat, inputs["src"], inputs["dst"])
    for im in in_maps:
        for i in (1, 2, 3, 4):
            im[f"w{i}s"] = np.asarray(inputs[f"w{i}s"]).astype(np.float32)
            im[f"w{i}n"] = np.asarray(inputs[f"w{i}n"]).astype(np.float32)
            im[f"b{i}"] = (
                np.asarray(inputs[f"b{i}"]).astype(np.float32).reshape(-1, P).T.copy()
            )

    key = "nc"
    if key not in _CACHE:
        _CACHE[key] = _build(structure)
    nc = _CACHE[key]

    res = run_bass_kernel_spmd(
        nc, in_maps, core_ids=list(range(NCORES)), trace=trace
    )
    outs = [res.results[c]["out"] for c in range(NCORES)]
    full = np.concatenate(outs, axis=0)  # [NPAD, H]
    result = full[rows_all]
    return result, res


def kernel(**inputs) -> np.ndarray:
    result, _ = _run(inputs, trace=False)
    return result


# revision 6
# speedup vs baseline: 2.2536x; 2.2460x over previous
"""GraphSAGE (4-layer) on 8 Trainium2 NeuronCores via Bass/Tile.

Sharding: nodes partitioned across 8 cores by destination (graph parallel).
Each core owns 6250 nodes (padded to 6272 = 49*128) and the edges whose dst
lands in its range. Per layer, the gather table (node features, or h@wn for
the matmul-first layer) is AllGathered so every core can gather arbitrary
source rows; the gather-mean is computed as one-hot matmuls on the
TensorEngine over dst-sorted edge tiles.

Aggregation is linear, so it always runs at width 128 for layers 1-3:
  L1: agg(feat)         @ w1n   (aggregate-first, 128 wide)
  L2: agg(h1 @ w2n)              (matmul-first, 128 wide)
  L3: agg(h2)           @ w3n   (aggregate-first, 128 wide)
  L4: agg(h3)           @ w4n   (256 wide)

Self-contained: hardcodes all shapes; builds + compiles the Bass program at
call time from the actual edge lists.
"""

import sys

sys.path.insert(0, "/opt/trn_rl_repo")

import numpy as np

import concourse.bacc as bacc
import concourse.bass as bass
import concourse.mybir as mybir
import concourse.tile as tile
from concourse.bass_utils import run_bass_kernel_spmd
from concourse.masks import make_identity

P = 128
NCORES = 8
N_NODES = 50000
PER = 6250  # real nodes per core
ROWS = 6272  # padded rows per core (49 * 128)
NB = ROWS // P  # 49 dst blocks per core
NPAD = NCORES * ROWS  # 50176 table rows
HALF = NPAD // 2  # 25088 rows per int16-addressable half
ZR = HALF - 1  # zero row index within each half (core 3/7 pad row 6271)
F_IN = 128
H = 256
MID = 128
F32 = mybir.dt.float32
I16 = mybir.dt.int16

# blocks per gather chunk, by gather width
CH_128 = 2
CH_256 = 1


def _preprocess(feat, src, dst):
    """Host-side sharding. Returns per-core input maps + common structure."""
    src = np.asarray(src).astype(np.int64)
    dst = np.asarray(dst).astype(np.int64)
    feat = np.asarray(feat).astype(np.float32)

    deg = np.bincount(dst, minlength=N_NODES).astype(np.float32)
    inv_deg = (1.0 / np.maximum(deg, 1.0)).astype(np.float32)

    src_row = (src // PER) * ROWS + (src % PER)
    half = (src_row // HALF).astype(np.int64)
    idx16 = (src_row % HALF).astype(np.int64)
    dst_core = dst // PER
    dst_local = dst % PER
    blk = dst_local // P
    ldst = dst_local % P

    # per-core, per-(block, half) edge counts
    counts = np.zeros((NCORES, NB, 2), np.int64)
    for c in range(NCORES):
        m = dst_core == c
        key = blk[m] * 2 + half[m]
        counts[c] = np.bincount(key, minlength=NB * 2).reshape(NB, 2)
    n_sub = np.maximum(1, -(-counts.max(axis=0) // P))  # [NB, 2] subtiles, >=1

    sub_base = np.zeros((NB, 2), np.int64)  # subtile offset within each half stream
    sub_base[1:, 0] = np.cumsum(n_sub[:-1, 0])
    sub_base[1:, 1] = np.cumsum(n_sub[:-1, 1])
    L_half = [int(n_sub[:, h].sum()) * P for h in range(2)]  # idxs per half
    col_base = np.zeros(NB, np.int64)  # S-stream column offset per block
    col_base[1:] = np.cumsum(n_sub.sum(axis=1))[:-1]
    S_TOT = int(n_sub.sum())

    in_maps = []
    for c in range(NCORES):
        m = np.nonzero(dst_core == c)[0]
        b_c, h_c, i_c, l_c = blk[m], half[m], idx16[m], ldst[m]
        order = np.lexsort((h_c, b_c))
        b_c, h_c, i_c, l_c = b_c[order], h_c[order], i_c[order], l_c[order]
        # rank within each (b, h) group
        key = b_c * 2 + h_c
        grp_cnt = np.bincount(key, minlength=NB * 2)
        grp_start = np.concatenate([[0], np.cumsum(grp_cnt)[:-1]])
        rank = np.arange(len(key)) - grp_start[key]
        s_idx = rank // P
        lane = rank % P

        idx_streams = []
        for h in range(2):
            arr = np.full(L_half[h], ZR, np.int64)
            sel = h_c == h
            pos = (sub_base[b_c[sel], h] + s_idx[sel]) * P + lane[sel]
            arr[pos] = i_c[sel]
            packed = arr.reshape(-1, 16).T.astype(np.int16)  # [16, L/16]
            idx_streams.append(np.tile(packed, (8, 1)).copy())

        ldst_arr = np.zeros((P, S_TOT), np.float32)
        col = col_base[b_c] + np.where(h_c == 0, s_idx, n_sub[b_c, 0] + s_idx)
        ldst_arr[lane, col] = l_c.astype(np.float32)

        feat_fm = np.zeros((P, ROWS), np.float32)
        feat_fm[:, :PER] = feat[c * PER : (c + 1) * PER].T
        inv_own = np.ones(ROWS, np.float32)
        inv_own[:PER] = inv_deg[c * PER : (c + 1) * PER]
        invdeg_rep = np.tile(inv_own[None, :], (P, 1)).copy()

        in_maps.append(
            {
                "idx0": idx_streams[0],
                "idx1": idx_streams[1],
                "ldst": ldst_arr,
                "feat_fm": feat_fm,
                "invdeg": invdeg_rep,
            }
        )

    feat_nm = np.zeros((NPAD, F_IN), np.float32)
    rows_all = (np.arange(N_NODES) // PER) * ROWS + (np.arange(N_NODES) % PER)
    feat_nm[rows_all] = feat
    maxsub = int(n_sub.sum(axis=1).max())
    iota = np.tile(np.arange(P, dtype=np.float32)[None, :], (P, maxsub)).copy()
    for im in in_maps:
        im["feat_nm"] = feat_nm
        im["iota"] = iota

    structure = dict(n_sub=n_sub, sub_base=sub_base, col_base=col_base, L_half=L_half, S_TOT=S_TOT, maxsub=maxsub)
    return in_maps, structure, rows_all


def _build(structure):
    n_sub = structure["n_sub"]
    sub_base = structure["sub_base"]
    col_base = structure["col_base"]
    L_half = structure["L_half"]
    S_TOT = structure["S_TOT"]

    nc = bacc.Bacc("TRN2", target_bir_lowering=False, debug=False, num_devices=NCORES)

    # --- I/O ---
    feat_nm = nc.dram_tensor("feat_nm", [NPAD, F_IN], F32, kind="ExternalInput")
    feat_fm_in = nc.dram_tensor("feat_fm", [P, ROWS], F32, kind="ExternalInput")
    idx_in = [
        nc.dram_tensor(f"idx{h}", [P, L_half[h] // 16], I16, kind="ExternalInput")
        for h in range(2)
    ]
    ldst_in = nc.dram_tensor("ldst", [P, S_TOT], F32, kind="ExternalInput")
    invdeg_in = nc.dram_tensor("invdeg", [P, ROWS], F32, kind="ExternalInput")
    MAXSUB = structure["maxsub"]
    iota_in = nc.dram_tensor("iota", [P, MAXSUB * P], F32, kind="ExternalInput")
    w_in = {}
    for name, shape in [
        ("w1s", [F_IN, H]), ("w1n", [F_IN, H]),
        ("w2s", [H, MID]), ("w2n", [H, MID]),
        ("w3s", [MID, H]), ("w3n", [MID, H]),
        ("w4s", [H, H]), ("w4n", [H, H]),
    ]:
        w_in[name] = nc.dram_tensor(name, shape, F32, kind="ExternalInput")
    b_in = {
        "b1": nc.dram_tensor("b1", [P, H // P], F32, kind="ExternalInput"),
        "b2": nc.dram_tensor("b2", [P, MID // P], F32, kind="ExternalInput"),
        "b3": nc.dram_tensor("b3", [P, H // P], F32, kind="ExternalInput"),
        "b4": nc.dram_tensor("b4", [P, H // P], F32, kind="ExternalInput"),
    }
    out = nc.dram_tensor("out", [ROWS, H], F32, kind="ExternalOutput")

    with tile.TileContext(nc) as tc:
        with (
            tc.tile_pool(name="const", bufs=1) as cpool,
            tc.tile_pool(name="hfm", bufs=4) as hpool,
            tc.tile_pool(name="gath", bufs=2) as gpool,
            tc.tile_pool(name="s", bufs=4) as spool,
            tc.tile_pool(name="small", bufs=4) as smpool,
            tc.tile_pool(name="pa", bufs=2, space="PSUM") as ppool_out,
            tc.tile_pool(name="pagg", bufs=2, space="PSUM") as ppool_agg,
            tc.tile_pool(name="dram", bufs=1, space="DRAM") as dpool,
        ):
            # ---- constants / weights to SBUF ----
            ident = cpool.tile([P, P], F32, name="ident")
            make_identity(nc, ident[:])
            iota_t = cpool.tile([P, MAXSUB * P], F32, name="iota_t")
            nc.sync.dma_start(iota_t[:], iota_in[:])
            ldst_t = cpool.tile([P, S_TOT], F32, name="ldst_t")
            nc.sync.dma_start(ldst_t[:], ldst_in[:])

            def load_w(name, kparts):
                """weight [K, M] split into kparts SBUF tiles of [128, M]."""
                tiles = []
                for k in range(kparts):
                    t = cpool.tile(
                        [P, w_in[name].shape[1]], F32, name=f"{name}_{k}"
                    )
                    nc.sync.dma_start(t[:], w_in[name][k * P : (k + 1) * P, :])
                    tiles.append(t)
                return tiles

            w = {
                "w1s": load_w("w1s", 1), "w1n": load_w("w1n", 1),
                "w2s": load_w("w2s", 2), "w2n": load_w("w2n", 2),
                "w3s": load_w("w3s", 1), "w3n": load_w("w3n", 1),
                "w4s": load_w("w4s", 2), "w4n": load_w("w4n", 2),
            }
            bia = {}
            for name, d in [("b1", H), ("b2", MID), ("b3", H), ("b4", H)]:
                t = cpool.tile([P, d // P], F32, name=f"{name}_t")
                nc.sync.dma_start(t[:], b_in[name][:])
                bia[name] = t

            # ---- DRAM scratch ----
            t2_own = dpool.tile([ROWS, MID], F32, name="t2_own")
            t2_full = dpool.tile([NPAD, MID], F32, addr_space="Shared", name="t2_full")
            t3_own = dpool.tile([ROWS, MID], F32, name="t3_own")
            t3_full = dpool.tile([NPAD, MID], F32, addr_space="Shared", name="t3_full")
            t4_own = dpool.tile([ROWS, H], F32, name="t4_own")
            t4_full = dpool.tile([NPAD, H], F32, addr_space="Shared", name="t4_full")

            # ---- persistent h (feature-major [128, ROWS]) ----
            feat_fm = hpool.tile([P, ROWS], F32, tag="hfm", name="feat_fm_t")
            nc.sync.dma_start(feat_fm[:], feat_fm_in[:])
            h1 = [hpool.tile([P, ROWS], F32, tag="hfm", name=f"h1_{i}") for i in range(2)]
            h2 = [hpool.tile([P, ROWS], F32, tag="hfm", name="h2_0")]
            h3 = [hpool.tile([P, ROWS], F32, tag="hfm", name=f"h3_{i}") for i in range(2)]

            def gather_chunk(table_half_aps, b0, nblk, fw, lname):
                """Gather all subtiles for blocks [b0, b0+nblk) of both halves.
                Returns (tiles, sub0) where tiles[h] is [128, nsub_h, fw]."""
                tiles = []
                for h in range(2):
                    ns = int(n_sub[b0 : b0 + nblk, h].sum())
                    g = gpool.tile(
                        [P, ns, fw], F32, tag=f"g{h}", name=f"g{lname}_{b0}_{h}"
                    )
                    nidx = ns * P
                    off16 = int(sub_base[b0, h]) * P // 16
                    ix = spool.tile(
                        [P, nidx // 16], I16, tag=f"ix{h}", name=f"ix{lname}_{b0}_{h}"
                    )
                    nc.sync.dma_start(ix[:], idx_in[h][:, off16 : off16 + nidx // 16])
                    # ucode is only stable up to ~384 idxs per call
                    for s0 in range(0, ns, 3):
                        sn = min(3, ns - s0)
                        nc.gpsimd.dma_gather(
                            out_ap=g[:, s0 : s0 + sn, :],
                            in_ap=table_half_aps[h],
                            idxs_ap=ix[:, s0 * 8 : (s0 + sn) * 8],
                            num_idxs=sn * P,
                            num_idxs_reg=sn * P,
                            elem_size=fw,
                            queue_num=0,
                        )
                    tiles.append(g)
                return tiles

            def scatter_block(gtiles, b0, b, fw, lname):
                """One-hot matmul aggregation for dst block b (chunk origin b0).
                Returns list of PSUM tiles [128, 128], one per 128-feat slice."""
                nfh = fw // P
                aggs = [
                    ppool_agg.tile(
                        [P, P], F32, tag=f"agg{fh}",
                        bufs=(2 if fh == 0 else 1),
                        name=f"agg{lname}_{b}_{fh}",
                    )
                    for fh in range(nfh)
                ]
                subs = []  # (half, local subtile index)
                for h in range(2):
                    for s in range(int(n_sub[b, h])):
                        subs.append((h, s))
                nsub_tot = len(subs)
                cb = int(col_base[b])
                S = spool.tile(
                    [P, nsub_tot, P], F32, tag="S", bufs=3, name=f"S{lname}_{b}"
                )
                nc.vector.tensor_tensor(
                    out=S[:],
                    in0=ldst_t[:, cb : cb + nsub_tot].to_broadcast([P, nsub_tot, P]),
                    in1=iota_t[:, : nsub_tot * P],
                    op=mybir.AluOpType.is_equal,
                )
                for j, (h, s) in enumerate(subs):
                    # subtile index within the chunk's gather tile
                    sloc = int(n_sub[b0:b, h].sum()) + s
                    for fh in range(nfh):
                        nc.tensor.matmul(
                            aggs[fh][:],
                            lhsT=gtiles[h][:, sloc, fh * P : (fh + 1) * P],
                            rhs=S[:, j, :],
                            start=(j == 0),
                            stop=(j == nsub_tot - 1),
                        )
                return aggs

            def layer(lname, table_aps, fw, wn, ws, h_prev, bias, dout, relu, h_out):
                """One GraphSAGE layer, dst-block streamed."""
                ch = CH_128 if fw == P else CH_256
                ndh = dout // P
                nkh = len(ws)  # K chunks of self path
                for b0 in range(0, NB, ch):
                    nblk = min(ch, NB - b0)
                    gtiles = gather_chunk(table_aps, b0, nblk, fw, lname)
                    for b in range(b0, b0 + nblk):
                        aggs = scatter_block(gtiles, b0, b, fw, lname)
                        # inv_deg scale: PSUM -> SBUF
                        invd = smpool.tile([P, P], F32, tag="invd", name=f"iv{lname}_{b}")
                        nc.sync.dma_start(invd[:], invdeg_in[:, b * P : (b + 1) * P])
                        aggs_s = []
                        for fh in range(len(aggs)):
                            a_s = smpool.tile(
                                [P, P], F32, tag=f"aggs{fh}", name=f"as{lname}_{b}_{fh}"
                            )
                            nc.vector.tensor_tensor(
                                out=a_s[:], in0=aggs[fh][:], in1=invd[:],
                                op=mybir.AluOpType.mult,
                            )
                            aggs_s.append(a_s)
                        for dh in range(ndh):
                            po = ppool_out.tile(
                                [P, P], F32, tag=f"out{dh}", name=f"po{lname}_{b}_{dh}"
                            )
                            first = True
                            # neighbor term
                            if wn is None:  # identity add (L2)
                                nc.tensor.matmul(
                                    po[:], lhsT=ident[:], rhs=aggs_s[0][:],
                                    start=first, stop=False,
                                )
                                first = False
                            else:
                                for fh in range(len(aggs_s)):
                                    nc.tensor.matmul(
                                        po[:],
                                        lhsT=wn[fh][:, dh * P : (dh + 1) * P],
                                        rhs=aggs_s[fh][:],
                                        start=first, stop=False,
                                    )
                                    first = False
                            # self term
                            for kh in range(nkh):
                                nc.tensor.matmul(
                                    po[:],
                                    lhsT=ws[kh][:, dh * P : (dh + 1) * P],
                                    rhs=h_prev[kh][:, b * P : (b + 1) * P],
                                    start=False, stop=(kh == nkh - 1),
                                )
                            dst_ap = h_out[dh][:, b * P : (b + 1) * P]
                            if relu:
                                nc.scalar.activation(
                                    dst_ap, po[:],
                                    mybir.ActivationFunctionType.Relu,
                                    bias=bias[:, dh : dh + 1],
                                )
                            else:
                                nc.vector.tensor_scalar_add(
                                    dst_ap, po[:], bias[:, dh : dh + 1]
                                )
                # zero the 22 pad columns so gather-table pad rows stay zero
                for dh in range(ndh):
                    nc.vector.memset(h_out[dh][:, PER:ROWS], 0.0)

            def allgather(own, full):
                nc.gpsimd.collective_compute(
                    "AllGather",
                    mybir.AluOpType.bypass,
                    replica_groups=[list(range(NCORES))],
                    ins=[own[:]],
                    outs=[full[:]],
                )

            def halves(t, fw):
                return [t[h * HALF : (h + 1) * HALF, :] for h in range(2)]

            # ================= Layer 1 =================
            layer(
                "l1", halves(feat_nm, F_IN), F_IN,
                wn=w["w1n"], ws=w["w1s"], h_prev=[feat_fm],
                bias=bia["b1"], dout=H, relu=True, h_out=h1,
            )

            # T2 = h1 @ w2n  (node-major), AllGather
            for rb in range(NB):
                pt = ppool_out.tile([P, MID], F32, tag="out0", name=f"t2_{rb}")
                for kh in range(2):
                    nc.tensor.matmul(
                        pt[:],
                        lhsT=h1[kh][:, rb * P : (rb + 1) * P],
                        rhs=w["w2n"][kh][:],
                        start=(kh == 0), stop=(kh == 1),
                    )
                st = smpool.tile([P, MID], F32, tag="stage", name=f"t2s_{rb}")
                nc.vector.tensor_copy(st[:], pt[:])
                nc.sync.dma_start(t2_own[rb * P : (rb + 1) * P, :], st[:])
            allgather(t2_own, t2_full)

            # ================= Layer 2 =================
            layer(
                "l2", halves(t2_full, MID), MID,
                wn=None, ws=w["w2s"], h_prev=h1,
                bias=bia["b2"], dout=MID, relu=True, h_out=h2,
            )

            # T3 = h2 (node-major via PE transpose), AllGather
            for rb in range(NB):
                pt = ppool_out.tile([P, P], F32, tag="out0", name=f"t3_{rb}")
                nc.tensor.transpose(pt[:], h2[0][:, rb * P : (rb + 1) * P], ident[:])
                st = smpool.tile([P, MID], F32, tag="stage", name=f"t3s_{rb}")
                nc.vector.tensor_copy(st[:], pt[:])
                nc.sync.dma_start(t3_own[rb * P : (rb + 1) * P, :], st[:])
            allgather(t3_own, t3_full)

            # ================= Layer 3 =================
            layer(
                "l3", halves(t3_full, MID), MID,
                wn=w["w3n"], ws=w["w3s"], h_prev=h2,
                bias=bia["b3"], dout=H, relu=True, h_out=h3,
            )

            # T4 = h3 (node-major via PE transpose), AllGather
            for rb in range(NB):
                st = smpool.tile([P, H], F32, tag="stage2", name=f"t4s_{rb}")
                for kh in range(2):
                    pt = ppool_out.tile([P, P], F32, tag=f"out{kh}", name=f"t4_{rb}_{kh}")
                    nc.tensor.transpose(
                        pt[:], h3[kh][:, rb * P : (rb + 1) * P], ident[:]
                    )
                    nc.vector.tensor_copy(st[:, kh * P : (kh + 1) * P], pt[:])
                nc.sync.dma_start(t4_own[rb * P : (rb + 1) * P, :], st[:])
            allgather(t4_own, t4_full)

            # ================= Layer 4 =================
            h4 = [hpool.tile([P, ROWS], F32, tag="hfm", name=f"h4_{i}") for i in range(2)]
            layer(
                "l4", halves(t4_full, H), H,
                wn=w["w4n"], ws=w["w4s"], h_prev=h3,
                bias=bia["b4"], dout=H, relu=False, h_out=h4,
            )

            # out = h4 transposed to node-major
            for rb in range(NB):
                st = smpool.tile([P, H], F32, tag="stage2", name=f"o_{rb}")
                for kh in range(2):
                    pt = ppool_out.tile([P, P], F32, tag=f"out{kh}", name=f"o_{rb}_{kh}")
                    nc.tensor.transpose(
                        pt[:], h4[kh][:, rb * P : (rb + 1) * P], ident[:]
                    )
                    nc.vector.tensor_copy(st[:, kh * P : (kh + 1) * P], pt[:])
                nc.sync.dma_start(out[rb * P : (rb + 1) * P, :], st[:])

    nc.compile()
    return nc


_CACHE = {}


def _run(inputs, trace=False):
    feat = inputs["feat"]
    in_maps, structure, rows_all = _preprocess(feat, inputs["src"], inputs["dst"])
    for im in in_maps:
        for i in (1, 2, 3, 4):
            im[f"w{i}s"] = np.asarray(inputs[f"w{i}s"]).astype(np.float32)
            im[f"w{i}n"] = np.asarray(inputs[f"w{i}n"]).astype(np.float32)
            im[f"b{i}"] = (
                np.asarray(inputs[f"b{i}"]).astype(np.float32).reshape(-1, P).T.copy()
            )

    key = "nc"
    if key not in _CACHE:
        _CACHE[key] = _build(structure)
    nc = _CACHE[key]

    res = run_bass_kernel_spmd(
        nc, in_maps, core_ids=list(range(NCORES)), trace=trace
    )
    outs = [res.results[c]["out"] for c in range(NCORES)]
    full = np.concatenate(outs, axis=0)  # [NPAD, H]
    result = full[rows_all]
    return result, res


def kernel(**inputs) -> np.ndarray:
    result, _ = _run(inputs, trace=False)
    return result
